# revision 10
# baseline (speedup 1.0000x reference)
"""Trainium2 Bass kernel for ConvexDisplacementUpdate (B=4, L=4096, D=256).

new_coords = alpha * softmax(10 * qhat @ khat^T) @ coords + (1-alpha) * coords
q = l2norm(latents @ Wq^T), k = l2norm(latents @ Wk^T)  (row-wise l2norm)

Strategy (flash-attention style; the [L, L] score matrix never touches HBM):
  - 8 cores = (4 batches) x (2 query halves of 2048 rows). Host rolls each
    core's per-batch data so its own query rows are always columns 0:2048
    of the transposed latents -> one SPMD program, no per-core control flow.
  - Scores are computed transposed, S^T[m, l] = k_m . qhat_l, with k left
    UN-normalized; the per-m factor 10/||k_m|| is a per-partition scale
    folded into the exp() activation.
  - softmax without max-subtraction (|scores| <= 10, exp is safe in fp32).
  - numerator and denominator come from one PE matmul per tile with the
    ones-augmented coords [x, y, 1] as the stationary operand, accumulated
    over all 32 m-tiles in PSUM.
  - final alpha-blend + division happen on host (B*L*2 elements, trivial).
"""

import numpy as np

B, L, D = 4, 4096, 256
HALF = L // 2  # 2048 query rows per core
NCORES = 8
INV_TEMP = 10.0

_CACHE = {}


def build_module(reps=1, use_f32r=True, phases=3):
    """Build + compile the SPMD Bass module (one program, 8 cores)."""
    from contextlib import ExitStack

    import concourse.bacc as bacc
    import concourse.mybir as mybir
    import concourse.tile as tile
    from concourse.bass import ts
    from concourse.masks import make_identity

    dt = mybir.dt
    f32 = dt.float32
    AF = mybir.ActivationFunctionType
    ALU = mybir.AluOpType

    fr = dt.float32r if use_f32r else f32

    def mm(ap):
        return ap

    nc = bacc.Bacc("TRN2", target_bir_lowering=False, debug=False,
                   num_devices=NCORES)

    latT = nc.dram_tensor("latT", [D, L], f32, kind="ExternalInput")
    wqT_d = nc.dram_tensor("wqT", [D, D], f32, kind="ExternalInput")
    wkT_d = nc.dram_tensor("wkT", [D, D], f32, kind="ExternalInput")
    caug_d = nc.dram_tensor("caug", [128, 3 * (L // 128)], f32,
                            kind="ExternalInput")
    pv_d = nc.dram_tensor("pv", [3, HALF], f32, kind="ExternalOutput")

    NLT = L // 128        # 32 m-tiles
    NQT = HALF // 128     # 16 q l-tiles
    NMB = L // 512        # 8 m-blocks
    NLB = HALF // 512     # 4 l-blocks

    with tile.TileContext(nc) as tc:
        for _rep in range(reps):
            with ExitStack() as ctx:
                persist = ctx.enter_context(tc.tile_pool(name="persist", bufs=1))

                # ---- load inputs ----
                lat = [persist.tile([128, L], fr, tag=f"lat{i}", name=f"lat{i}") for i in range(2)]
                for i in range(2):
                    # own-query half first so phase 1q can start early
                    for c in range(4):
                        nc.sync.dma_start(out=lat[i][:, ts(c, 1024)],
                                          in_=latT[i * 128:(i + 1) * 128,
                                                   ts(c, 1024)].bitcast(fr))
                wq = [persist.tile([128, D], fr, tag=f"wq{i}", name=f"wq{i}") for i in range(2)]
                wk = [persist.tile([128, D], fr, tag=f"wk{i}", name=f"wk{i}") for i in range(2)]
                for i in range(2):
                    nc.sync.dma_start(out=wq[i], in_=wqT_d[i * 128:(i + 1) * 128, :].bitcast(fr))
                    nc.sync.dma_start(out=wk[i], in_=wkT_d[i * 128:(i + 1) * 128, :].bitcast(fr))
                caug = persist.tile([128, 3 * NLT], fr, tag="caug")
                nc.sync.dma_start(out=caug, in_=caug_d[:, :].bitcast(fr))

                ident = persist.tile([128, 128], f32, tag="ident")
                make_identity(nc, ident)

                # ---- phase 1q: qhatT [e, l] for own 2048 rows ----
                qT = [persist.tile([128, HALF], fr, tag=f"qT{i}", name=f"qT{i}") for i in range(2)]
                with ExitStack() as p1q:
                    qle_ps = p1q.enter_context(
                        tc.tile_pool(name="qle_ps", bufs=3, space="PSUM"))
                    tp_ps = p1q.enter_context(
                        tc.tile_pool(name="tp_ps", bufs=3, space="PSUM"))
                    sm = p1q.enter_context(tc.tile_pool(name="q_small", bufs=4))
                    qh_pool = p1q.enter_context(tc.tile_pool(name="qhat", bufs=3))
                    for lt in range(NQT):
                        qle = qle_ps.tile([128, D], f32, tag="qle")
                        nc.tensor.matmul(qle, mm(lat[0][:, ts(lt, 128)]),
                                         mm(wq[0]), start=True, stop=False)
                        nc.tensor.matmul(qle, mm(lat[1][:, ts(lt, 128)]),
                                         mm(wq[1]), start=False, stop=True)
                        q_sb = qh_pool.tile([128, D], f32, tag="q_sb")
                        nc.scalar.copy(out=q_sb, in_=qle)
                        sq = sm.tile([128, D], f32, tag="sqj")
                        ssq = sm.tile([128, 1], f32, tag="ssq")
                        nc.vector.tensor_mul(sq, q_sb, q_sb)
                        nc.vector.reduce_sum(ssq, sq, axis=mybir.AxisListType.X)
                        nrm = sm.tile([128, 1], f32, tag="nrm")
                        nc.scalar.activation(nrm, ssq, AF.Sqrt)
                        inv = sm.tile([128, 1], f32, tag="inv")
                        nc.vector.reciprocal(inv, nrm)
                        qhat = qh_pool.tile([128, D], f32, tag="qhat")
                        nc.vector.tensor_scalar_mul(qhat, q_sb, inv)
                        for et in range(2):
                            tp = tp_ps.tile([128, 128], f32, tag="tp")
                            nc.tensor.transpose(tp, qhat[:, ts(et, 128)], ident)
                            nc.vector.tensor_copy(out=qT[et][:, ts(lt, 128)], in_=tp)

                if phases < 2:
                    with tc.tile_pool(name="dbg", bufs=1) as dbg:
                        dtile = dbg.tile([3, HALF], f32, name="dtile")
                        nc.vector.tensor_copy(out=dtile, in_=qT[0][0:3, :])
                        nc.sync.dma_start(out=pv_d[:, :], in_=dtile)
                    continue
                # ---- phase 1k: kT_raw [e, m] + inv_k (10/||k_m||) ----
                kT = [persist.tile([128, L], fr, tag=f"kT{i}", name=f"kT{i}") for i in range(2)]
                inv_row = persist.tile([1, L], f32, tag="inv_row")
                inv_kT = persist.tile([128, NLT], f32, tag="inv_kT")
                with ExitStack() as p1k:
                    kp_ps = p1k.enter_context(
                        tc.tile_pool(name="kp_ps", bufs=3, space="PSUM"))
                    ssq_ps = p1k.enter_context(
                        tc.tile_pool(name="ssq_ps", bufs=2, space="PSUM"))
                    sq_pool = p1k.enter_context(tc.tile_pool(name="k_sq", bufs=3))
                    ksm = p1k.enter_context(tc.tile_pool(name="k_small", bufs=3))
                    dram = p1k.enter_context(
                        tc.tile_pool(name="bounce", bufs=1, space="DRAM"))
                    for mb in range(NMB):
                        sqs = []
                        for et in range(2):
                            kp = kp_ps.tile([128, 512], f32, tag="kp")
                            nc.tensor.matmul(kp, mm(wk[0][:, ts(et, 128)]),
                                             mm(lat[0][:, ts(mb, 512)]),
                                             start=True, stop=False)
                            nc.tensor.matmul(kp, mm(wk[1][:, ts(et, 128)]),
                                             mm(lat[1][:, ts(mb, 512)]),
                                             start=False, stop=True)
                            nc.scalar.copy(out=kT[et][:, ts(mb, 512)], in_=kp)
                            sq = sq_pool.tile([128, 512], fr, tag="ksq")
                            nc.vector.tensor_mul(sq, kT[et][:, ts(mb, 512)],
                                                 kT[et][:, ts(mb, 512)])
                            sqs.append(sq)
                        ssqp = ssq_ps.tile([1, 512], f32, tag="kssq")
                        nc.tensor.matmul(ssqp, caug[:, 2:3], mm(sqs[0]),
                                         start=True, stop=False)
                        nc.tensor.matmul(ssqp, caug[:, 2:3], mm(sqs[1]),
                                         start=False, stop=True)
                        # ||k_m|| / 10 = sqrt(ssq * 0.01)
                        nrm = ksm.tile([1, 512], f32, tag="knrm")
                        nc.scalar.activation(nrm, ssqp, AF.Sqrt,
                                             scale=1.0 / (INV_TEMP * INV_TEMP))
                        nc.vector.reciprocal(inv_row[:, ts(mb, 512)], nrm)
                    # [1, L] -> [128, L/128] transposed layout via DRAM bounce
                    tmp = dram.tile([L], f32, tag="bnc")
                    nc.sync.dma_start(out=tmp.rearrange("(a f) -> a f", a=1),
                                      in_=inv_row[0:1, :])
                    nc.sync.dma_start(out=inv_kT,
                                      in_=tmp.rearrange("(t p) -> p t", p=128))

                if phases < 3:
                    with tc.tile_pool(name="dbg", bufs=1) as dbg:
                        dtile = dbg.tile([3, HALF], f32, name="dtile")
                        nc.vector.tensor_copy(out=dtile, in_=kT[0][0:3, 0:HALF])
                        nc.sync.dma_start(out=pv_d[:, :], in_=dtile)
                    continue
                # ---- phase 2: scores^T -> exp -> [coords|1]^T @ P^T ----
                with ExitStack() as p2:
                    pv_ps = p2.enter_context(
                        tc.tile_pool(name="pv_ps", bufs=NLB, space="PSUM"))
                    s_ps = p2.enter_context(
                        tc.tile_pool(name="s_ps", bufs=3, space="PSUM"))
                    p_pool = p2.enter_context(tc.tile_pool(name="p_sb", bufs=4))
                    pv = [pv_ps.tile([3, 512], f32, tag="pv", name=f"pv{lb}")
                          for lb in range(NLB)]
                    for t in range(NLT):
                        for lb in range(NLB):
                            sp = s_ps.tile([128, 512], f32, tag="sp")
                            nc.tensor.matmul(sp, mm(kT[0][:, ts(t, 128)]),
                                             mm(qT[0][:, ts(lb, 512)]),
                                             start=True, stop=False)
                            nc.tensor.matmul(sp, mm(kT[1][:, ts(t, 128)]),
                                             mm(qT[1][:, ts(lb, 512)]),
                                             start=False, stop=True)
                            p = p_pool.tile([128, 512], fr, tag="p")
                            nc.scalar.activation(p, sp, AF.Exp,
                                                 scale=inv_kT[:, t:t + 1])
                            nc.tensor.matmul(pv[lb], mm(caug[:, ts(t, 3)]),
                                             mm(p), start=(t == 0),
                                             stop=(t == NLT - 1))
                    out_sb = p2.enter_context(tc.tile_pool(name="out_sb", bufs=2))
                    for lb in range(NLB):
                        ot = out_sb.tile([3, 512], f32, tag="ot")
                        nc.vector.tensor_copy(out=ot, in_=pv[lb])
                        nc.sync.dma_start(out=pv_d[:, ts(lb, 512)], in_=ot)

    nc.compile()
    return nc


def _get_module():
    if "nc" not in _CACHE:
        _CACHE["nc"] = build_module()
    return _CACHE["nc"]


def make_in_maps(latents, current_coords, Wq, Wk):
    """Per-core input dicts. Core c -> batch c//2, query half c%2 (rolled
    so own query rows are always columns 0:2048)."""
    latents = np.asarray(latents, np.float32)
    coords = np.asarray(current_coords, np.float32)
    wqT = np.ascontiguousarray(np.asarray(Wq, np.float32).T)
    wkT = np.ascontiguousarray(np.asarray(Wk, np.float32).T)
    in_maps = []
    for c in range(NCORES):
        b, h = divmod(c, 2)
        lat_b = np.roll(latents[b], -HALF * h, axis=0)
        coo_b = np.roll(coords[b], -HALF * h, axis=0)
        aug = np.concatenate([coo_b, np.ones((L, 1), np.float32)], axis=1)
        caug = np.ascontiguousarray(
            aug.reshape(L // 128, 128, 3).transpose(1, 0, 2).reshape(128, -1))
        in_maps.append({
            "latT": np.ascontiguousarray(lat_b.T),
            "wqT": wqT,
            "wkT": wkT,
            "caug": caug,
        })
    return in_maps


def postprocess(results, current_coords, alpha):
    """Assemble (new_coords, displacement) from per-core pv = [num_x; num_y; den]."""
    coords = np.asarray(current_coords, np.float32)
    new_coords = np.empty((B, L, 2), np.float32)
    for c in range(NCORES):
        b, h = divmod(c, 2)
        pv = results[c]["pv"]
        wc = (pv[0:2, :] / pv[2:3, :]).T  # [2048, 2] = (W @ coords) rows
        rows = slice(h * HALF, (h + 1) * HALF)
        new_coords[b, rows] = alpha * wc + (1.0 - alpha) * coords[b, rows]
    displacement = new_coords - coords
    return new_coords, displacement


def kernel(latents, current_coords, Wq, Wk, alpha_raw, layer_idx=None):
    from concourse.bass_utils import run_bass_kernel_spmd

    nc = _get_module()
    in_maps = make_in_maps(latents, current_coords, Wq, Wk)
    res = run_bass_kernel_spmd(nc, in_maps, list(range(NCORES)))
    alpha = np.float32(1.0 / (1.0 + np.exp(-np.float64(np.asarray(alpha_raw)))))
    return postprocess(res.results, current_coords, alpha)
